# revision 7
# baseline (speedup 1.0000x reference)
"""VQ codebook forward kernel for Trainium2 (Bass/Tile), SPMD over 8 NeuronCores.

Problem (nn_BOB_87600152969626, arch vq_codebook):
  feat (32, 384, 28, 28) fp32, codebook (2048, 384) fp32.
  Returns (f, q_feat, assignment, distance) exactly like the jax reference:
    f          (32, 28, 28, 384)   = transpose(feat)
    q_feat     (32, 384, 28, 28)   = nearest (L2) normalized-codebook row per token
    assignment (32, 2048, 28, 28)  = softmax(-distance/0.1) over K
    distance   (25088, 2048)       = squared euclidean dists of normalized tokens/codes

Sharding: data-parallel over batch — core m handles batches [4m, 4m+4), the
(2048, 384) codebook is replicated.  No collectives needed (forward only).

Per-core math (tokens on psum partitions, K on free dim):
  m[t,k]   = sum_c feat[c,t] * cn[c,k]        (raw tokens x normalized codes, fp32 PE)
  dist     = (-2/nx[t]) * m + (sumsq_fn[t] + 1.0)     [se_k ~ 1.0 +- 1e-7 dropped]
  e        = exp((20/nx[t]) * m - 10*sumsq_fn[t])     (ACT, fused from psum, accum -> row sums)
  prob     = e / sum_k e                               (in-place tensor_scalar)
  idx      = first index with dist == min_k dist       (reduce-min + max_index)
  q        = cn[idx]                                   (indirect DMA gather)
assignment and q_feat are produced via PE-transposes into their (K|C, hw) layouts.
"""

import math
import numpy as np

import concourse.bacc as bacc
import concourse.bass as bass
import concourse.mybir as mybir
import concourse.tile as tile
from concourse.bass import AP
from concourse.bass_utils import run_bass_kernel_spmd
from concourse.masks import make_identity

dt = mybir.dt
Alu = mybir.AluOpType
Act = mybir.ActivationFunctionType

N_CORES = 8
B_FULL = 32
C = 384            # channels; 3 chunks of 128
CC = 3
K = 2048           # codebook size; 4 K-tiles of 512, 16 k-chunks of 128
KT = 4
KC = 16
H = W = 28
HW = 784           # tokens per batch
TT = 112           # token tile (7 per batch, no raggedness)
NJ = 7
RSQRT_MAGIC = 0x5F3759DF


def _rsqrt_newton(nc, pool, x_ap, p, f, iters=3):
    """y ~= 1/sqrt(x) to ~1-2 ulp: bit-trick seed + newton iterations (DVE only).

    Avoids the banned ACT Rsqrt and the loose ACT Sqrt table.
    """
    ti = pool.tile([p, f], dt.int32)
    # i >> 1
    nc.vector.tensor_scalar(
        out=ti[:], in0=x_ap.bitcast(dt.int32), scalar1=1, scalar2=None,
        op0=Alu.logical_shift_right,
    )
    # MAGIC - (i >> 1)  ==  (i>>1) * -1 + MAGIC
    nc.vector.tensor_scalar(
        out=ti[:], in0=ti[:], scalar1=-1, scalar2=RSQRT_MAGIC,
        op0=Alu.mult, op1=Alu.add,
    )
    y = pool.tile([p, f], dt.float32)
    nc.vector.tensor_copy(out=y[:], in_=ti[:].bitcast(dt.float32))
    t = pool.tile([p, f], dt.float32)
    for _ in range(iters):
        nc.vector.tensor_tensor(out=t[:], in0=y[:], in1=y[:], op=Alu.mult)
        nc.vector.tensor_tensor(out=t[:], in0=t[:], in1=x_ap, op=Alu.mult)
        nc.vector.tensor_scalar(
            out=t[:], in0=t[:], scalar1=-0.5, scalar2=1.5, op0=Alu.mult, op1=Alu.add
        )
        nc.vector.tensor_tensor(out=y[:], in0=y[:], in1=t[:], op=Alu.mult)
    return y


def build_nc(n_batch=4):
    nc = bacc.Bacc("TRN2", target_bir_lowering=False, debug=False, num_devices=N_CORES)
    n_tok = n_batch * HW

    feat = nc.dram_tensor("feat", (n_batch, C, H, W), dt.float32, kind="ExternalInput")
    cb = nc.dram_tensor("codebook", (K, C), dt.float32, kind="ExternalInput")
    f_out = nc.dram_tensor("f_out", (n_batch, HW, C), dt.float32, kind="ExternalOutput")
    qf_out = nc.dram_tensor("qf_out", (n_batch, C, HW), dt.float32, kind="ExternalOutput")
    asg_out = nc.dram_tensor("asg_out", (n_batch, K, HW), dt.float32, kind="ExternalOutput")
    dist_out = nc.dram_tensor("dist_out", (n_tok, K), dt.float32, kind="ExternalOutput")

    feat_ap = feat[:, :, :, :].rearrange("b c h w -> b c (h w)")

    with tile.TileContext(nc) as tc:
        with (
            tc.tile_pool(name="const", bufs=1) as const,
            tc.tile_pool(name="prep", bufs=1) as prep,
            tc.tile_pool(name="small", bufs=2) as small,
            tc.tile_pool(name="fp", bufs=2) as fp,
            tc.tile_pool(name="ft", bufs=2) as ftp,
            tc.tile_pool(name="ed", bufs=2) as ed,
            tc.tile_pool(name="qp", bufs=3) as qp,
            tc.tile_pool(name="asg", bufs=2) as asgp,
            tc.tile_pool(name="qft", bufs=2) as qftp,
            tc.tile_pool(name="dram", bufs=1, space="DRAM") as dramp,
            tc.tile_pool(name="mm_ps", bufs=4, space="PSUM") as mmps,
            tc.tile_pool(name="tp_ps", bufs=3, space="PSUM") as tpps,
        ):
            ident = const.tile([128, 128], dt.float32)
            make_identity(nc, ident[:])

            # ---------------- codebook prep ----------------
            cb_sb = prep.tile([128, KC, C], dt.float32, tag="cb")
            nc.sync.dma_start(
                out=cb_sb[:], in_=cb[:, :].rearrange("(n p) c -> p n c", p=128)
            )
            ne2 = const.tile([128, KC], dt.float32)
            scr_cb = prep.tile([128, C], dt.float32, tag="scr")
            for n in range(KC):
                nc.scalar.activation(
                    out=scr_cb[:], in_=cb_sb[:, n, :], func=Act.Square,
                    accum_out=ne2[:, n : n + 1],
                )
            rs_e = _rsqrt_newton(nc, const, ne2[:], 128, KC)
            cn_sb = prep.tile([128, KC, C], dt.float32, tag="cn")
            for n in range(KC):
                nc.scalar.activation(
                    out=cn_sb[:, n, :], in_=cb_sb[:, n, :], func=Act.Identity,
                    scale=rs_e[:, n : n + 1],
                )
            # normalized codebook back to DRAM as the gather table
            cn_dram = dramp.tile([K, C], dt.float32)
            nc.sync.dma_start(
                out=cn_dram[:, :].rearrange("(n p) c -> p n c", p=128), in_=cn_sb[:]
            )
            # transposed codebook (c, K) for the PE: 3 chunks of (128, 2048)
            cnT = const.tile([128, CC, K], dt.float32)
            for n in range(KC):
                for cc in range(CC):
                    tp = tpps.tile([128, 128], dt.float32, tag="tp")
                    nc.tensor.transpose(
                        tp[:], cn_sb[:, n, cc * 128 : (cc + 1) * 128], ident[:]
                    )
                    nc.scalar.copy(out=cnT[:, cc, n * 128 : (n + 1) * 128], in_=tp[:])

            # ---------------- per batch ----------------
            for b in range(n_batch):
                f_sb = fp.tile([128, CC, HW], dt.float32)
                nc.sync.dma_start(
                    out=f_sb[:],
                    in_=feat_ap[b].rearrange("(cc p) hw -> p cc hw", p=128),
                )

                # f transpose (tokens, c) + token norms
                fT = ftp.tile([TT, NJ, C], dt.float32)
                nsq = small.tile([TT, NJ], dt.float32, tag="nsq")
                scr_f = small.tile([TT, C], dt.float32, tag="scrf")
                for j in range(NJ):
                    ft_ps = tpps.tile([TT, C], dt.float32, tag="tp")
                    for cc in range(CC):
                        nc.tensor.transpose(
                            ft_ps[:, cc * 128 : (cc + 1) * 128],
                            f_sb[:, cc, j * TT : (j + 1) * TT],
                            ident[:],
                        )
                    nc.scalar.copy(out=fT[:, j, :], in_=ft_ps[:])
                    nc.scalar.activation(
                        out=scr_f[:], in_=ft_ps[:], func=Act.Square,
                        accum_out=nsq[:, j : j + 1],
                    )
                nc.sync.dma_start(
                    out=f_out[b].rearrange("(j p) c -> p j c", p=TT), in_=fT[:]
                )

                rs_x = _rsqrt_newton(nc, small, nsq[:], TT, NJ)  # 1/nx
                sx = small.tile([TT, NJ], dt.float32, tag="sx")
                t1 = small.tile([TT, NJ], dt.float32, tag="t1")
                nc.vector.tensor_tensor(out=t1[:], in0=rs_x[:], in1=rs_x[:], op=Alu.mult)
                nc.vector.tensor_tensor(out=sx[:], in0=nsq[:], in1=t1[:], op=Alu.mult)
                s1 = small.tile([TT, NJ], dt.float32, tag="s1")   # -2/nx
                s2 = small.tile([TT, NJ], dt.float32, tag="s2")   # sx + 1 (se ~ 1)
                sE = small.tile([TT, NJ], dt.float32, tag="sE")   # 20/nx
                bE = small.tile([TT, NJ], dt.float32, tag="bE")   # -10*sx
                nc.vector.tensor_scalar(out=s1[:], in0=rs_x[:], scalar1=-2.0, scalar2=None, op0=Alu.mult)
                nc.vector.tensor_scalar(out=s2[:], in0=sx[:], scalar1=1.0, scalar2=None, op0=Alu.add)
                nc.vector.tensor_scalar(out=sE[:], in0=rs_x[:], scalar1=20.0, scalar2=None, op0=Alu.mult)
                nc.vector.tensor_scalar(out=bE[:], in0=sx[:], scalar1=-10.0, scalar2=None, op0=Alu.mult)

                qfT = qftp.tile([128, CC, HW], dt.float32)

                for j in range(NJ):
                    js = slice(j * TT, (j + 1) * TT)
                    e_sb = ed.tile([TT, K], dt.float32, tag="e")
                    dist_sb = ed.tile([TT, K], dt.float32, tag="d")
                    es4 = small.tile([TT, KT], dt.float32, tag="es4")

                    m_ps = [
                        mmps.tile([TT, 512], dt.float32, tag="mm", name=f"m_ps{kt}")
                        for kt in range(KT)
                    ]
                    for cc in range(CC):
                        for kt in range(KT):
                            nc.tensor.matmul(
                                m_ps[kt][:],
                                lhsT=f_sb[:, cc, js],
                                rhs=cnT[:, cc, kt * 512 : (kt + 1) * 512],
                                start=(cc == 0),
                                stop=(cc == CC - 1),
                            )
                    for kt in range(KT):
                        ks = slice(kt * 512, (kt + 1) * 512)
                        nc.scalar.activation(
                            out=e_sb[:, ks], in_=m_ps[kt][:], func=Act.Exp,
                            scale=sE[:, j : j + 1], bias=bE[:, j : j + 1],
                            accum_out=es4[:, kt : kt + 1],
                        )
                        nc.scalar.activation(
                            out=dist_sb[:, ks], in_=m_ps[kt][:], func=Act.Identity,
                            scale=s1[:, j : j + 1], bias=s2[:, j : j + 1],
                        )
                    row0 = b * HW + j * TT
                    nc.sync.dma_start(out=dist_out[row0 : row0 + TT, :], in_=dist_sb[:])

                    # softmax denominator and in-place prob
                    esum = small.tile([TT, 1], dt.float32, tag="esum")
                    nc.vector.reduce_sum(out=esum[:], in_=es4[:], axis=mybir.AxisListType.X)
                    r = small.tile([TT, 1], dt.float32, tag="r")
                    nc.vector.reciprocal(out=r[:], in_=esum[:])
                    nc.vector.tensor_scalar(
                        out=e_sb[:], in0=e_sb[:], scalar1=r[:, :1], scalar2=None, op0=Alu.mult
                    )

                    # argmin (first index, matching jnp.argmin) on dist
                    dmin = small.tile([TT, 1], dt.float32, tag="dmin")
                    nc.vector.tensor_reduce(
                        out=dmin[:], in_=dist_sb[:], axis=mybir.AxisListType.X, op=Alu.min
                    )
                    dmin8 = small.tile([TT, 8], dt.float32, tag="dmin8")
                    nc.vector.tensor_copy(out=dmin8[:], in_=dmin[:, :1].to_broadcast([TT, 8]))
                    idx8 = small.tile([TT, 8], dt.uint32, tag="idx8")
                    nc.vector.max_index(out=idx8[:], in_max=dmin8[:], in_values=dist_sb[:])

                    # gather nearest codes and transpose into (c, hw)
                    q_sb = qp.tile([TT, C], dt.float32)
                    nc.gpsimd.indirect_dma_start(
                        out=q_sb[:],
                        out_offset=None,
                        in_=cn_dram[:, :],
                        in_offset=bass.IndirectOffsetOnAxis(ap=idx8[:, :1], axis=0),
                    )
                    qt_ps = tpps.tile([128, CC * TT], dt.float32, tag="tp")
                    for cc in range(CC):
                        nc.tensor.transpose(
                            qt_ps[:, cc * TT : (cc + 1) * TT],
                            q_sb[:, cc * 128 : (cc + 1) * 128],
                            ident[:TT, :TT],
                        )
                    nc.scalar.copy(
                        out=qfT[:, :, js],
                        in_=qt_ps[:].rearrange("p (cc t) -> p cc t", cc=CC),
                    )

                    # prob transpose into (k, hw): 16 chunks in groups of 4
                    if j % 2 == 0:
                        asgT = asgp.tile([128, KC, 2 * TT], dt.float32)
                    half = (j % 2) * TT
                    for g in range(KT):
                        tp_ps = tpps.tile([128, 4 * TT], dt.float32, tag="tp")
                        for u in range(4):
                            kc = 4 * g + u
                            nc.tensor.transpose(
                                tp_ps[:, u * TT : (u + 1) * TT],
                                e_sb[:, kc * 128 : (kc + 1) * 128],
                                ident[:TT, :TT],
                            )
                        nc.scalar.copy(
                            out=asgT[:, 4 * g : 4 * g + 4, half : half + TT],
                            in_=tp_ps[:].rearrange("p (k t) -> p k t", k=4),
                        )
                    if j % 2 == 1 or j == NJ - 1:
                        jp0 = (j // 2) * 2 * TT
                        width = TT if j == NJ - 1 and NJ % 2 == 1 else 2 * TT
                        nc.sync.dma_start(
                            out=asg_out[b][:, jp0 : jp0 + width].rearrange(
                                "(kc p) t -> p kc t", p=128
                            ),
                            in_=asgT[:, :, :width],
                        )

                nc.sync.dma_start(
                    out=qf_out[b].rearrange("(cc p) hw -> p cc hw", p=128), in_=qfT[:]
                )

    nc.compile()
    return nc


_NC_CACHE = {}


def _get_nc(n_batch):
    if n_batch not in _NC_CACHE:
        _NC_CACHE[n_batch] = build_nc(n_batch)
    return _NC_CACHE[n_batch]


def kernel(feat: np.ndarray, codebook: np.ndarray, _trace=False):
    feat = np.ascontiguousarray(np.asarray(feat, dtype=np.float32))
    codebook = np.ascontiguousarray(np.asarray(codebook, dtype=np.float32))
    b_full = feat.shape[0]
    bpc = b_full // N_CORES
    nc = _get_nc(bpc)

    in_maps = [
        {"feat": feat[m * bpc : (m + 1) * bpc], "codebook": codebook}
        for m in range(N_CORES)
    ]
    res = run_bass_kernel_spmd(nc, in_maps, list(range(N_CORES)), trace=_trace)

    f = np.concatenate([r["f_out"] for r in res.results], axis=0)
    qf = np.concatenate([r["qf_out"] for r in res.results], axis=0)
    asg = np.concatenate([r["asg_out"] for r in res.results], axis=0)
    dist = np.concatenate([r["dist_out"] for r in res.results], axis=0)

    f = f.reshape(b_full, H, W, C)
    qf = qf.reshape(b_full, C, H, W)
    asg = asg.reshape(b_full, K, H, W)
    out = (f, qf, asg, dist)
    if _trace:
        return out, res
    return out


# revision 13
# speedup vs baseline: 10.4390x; 10.4390x over previous
"""VQ codebook forward kernel for Trainium2 (Bass/Tile), SPMD over 8 NeuronCores.

Problem (nn_BOB_87600152969626, arch vq_codebook):
  feat (32, 384, 28, 28) fp32, codebook (2048, 384) fp32.
  Returns (f, q_feat, assignment, distance) exactly like the jax reference:
    f          (32, 28, 28, 384)   = transpose(feat)
    q_feat     (32, 384, 28, 28)   = nearest (L2) normalized-codebook row per token
    assignment (32, 2048, 28, 28)  = softmax(-distance/0.1) over K
    distance   (25088, 2048)       = squared euclidean dists of normalized tokens/codes

Sharding: data-parallel over batch — core m handles batches [4m, 4m+4), the
(2048, 384) codebook is replicated.  No collectives needed (forward only).

Per-core math (tokens on psum partitions, K on free dim):
  m[t,k]   = sum_c feat[c,t] * cn[c,k]        (raw tokens x normalized codes, fp32 PE)
  dist     = (-2/nx[t]) * m + (sumsq_fn[t] + 1.0)     [se_k ~ 1.0 +- 1e-7 dropped]
  e        = exp((20/nx[t]) * m - 10*sumsq_fn[t])     (ACT, fused from psum, accum -> row sums)
  prob     = e / sum_k e                               (in-place tensor_scalar)
  idx      = first index with dist == min_k dist       (reduce-min + max_index)
  q        = cn[idx]                                   (indirect DMA gather)
assignment and q_feat are produced via PE-transposes into their (K|C, hw) layouts.
"""

import math
import numpy as np

import concourse.bacc as bacc
import concourse.bass as bass
import concourse.mybir as mybir
import concourse.tile as tile
from concourse.bass import AP
from concourse.bass_utils import run_bass_kernel_spmd
from concourse.masks import make_identity

dt = mybir.dt
Alu = mybir.AluOpType
Act = mybir.ActivationFunctionType

N_CORES = 8
B_FULL = 32
C = 384            # channels; 3 chunks of 128
CC = 3
K = 2048           # codebook size; 4 K-tiles of 512, 16 k-chunks of 128
KT = 4
KC = 16
H = W = 28
HW = 784           # tokens per batch
TT = 112           # token tile (7 per batch, no raggedness)
NJ = 7
RSQRT_MAGIC = 0x5F3759DF


def _rsqrt_newton(nc, pool, x_ap, p, f, iters=3):
    """y ~= 1/sqrt(x) to ~1-2 ulp: bit-trick seed + newton iterations (DVE only).

    Avoids the banned ACT Rsqrt and the loose ACT Sqrt table.
    """
    ti = pool.tile([p, f], dt.int32)
    # i >> 1
    nc.vector.tensor_scalar(
        out=ti[:], in0=x_ap.bitcast(dt.int32), scalar1=1, scalar2=None,
        op0=Alu.logical_shift_right,
    )
    # MAGIC - (i >> 1)  ==  (i>>1) * -1 + MAGIC
    nc.vector.tensor_scalar(
        out=ti[:], in0=ti[:], scalar1=-1, scalar2=RSQRT_MAGIC,
        op0=Alu.mult, op1=Alu.add,
    )
    y = pool.tile([p, f], dt.float32)
    nc.vector.tensor_copy(out=y[:], in_=ti[:].bitcast(dt.float32))
    t = pool.tile([p, f], dt.float32)
    for _ in range(iters):
        nc.vector.tensor_tensor(out=t[:], in0=y[:], in1=y[:], op=Alu.mult)
        nc.vector.tensor_tensor(out=t[:], in0=t[:], in1=x_ap, op=Alu.mult)
        nc.vector.tensor_scalar(
            out=t[:], in0=t[:], scalar1=-0.5, scalar2=1.5, op0=Alu.mult, op1=Alu.add
        )
        nc.vector.tensor_tensor(out=y[:], in0=y[:], in1=t[:], op=Alu.mult)
    return y


def build_nc(n_batch=4):
    nc = bacc.Bacc("TRN2", target_bir_lowering=False, debug=False, num_devices=N_CORES)
    n_tok = n_batch * HW

    feat = nc.dram_tensor("feat", (n_batch, C, H, W), dt.float32, kind="ExternalInput")
    cb = nc.dram_tensor("codebook", (K, C), dt.float32, kind="ExternalInput")
    f_out = nc.dram_tensor("f_out", (n_batch, HW, C), dt.float32, kind="ExternalOutput")
    qf_out = nc.dram_tensor("qf_out", (n_batch, C, HW), dt.float32, kind="ExternalOutput")
    asg_out = nc.dram_tensor("asg_out", (n_batch, K, HW), dt.float32, kind="ExternalOutput")
    dist_out = nc.dram_tensor("dist_out", (n_tok, K), dt.float32, kind="ExternalOutput")

    feat_ap = feat[:, :, :, :].rearrange("b c h w -> b c (h w)")

    with tile.TileContext(nc) as tc:
        with (
            tc.tile_pool(name="const", bufs=1) as const,
            tc.tile_pool(name="prep", bufs=1) as prep,
            tc.tile_pool(name="small", bufs=2) as small,
            tc.tile_pool(name="vecs", bufs=4) as vecs,
            tc.tile_pool(name="fp", bufs=4) as fp,
            tc.tile_pool(name="ft", bufs=2) as ftp,
            tc.tile_pool(name="ed", bufs=3) as ed,
            tc.tile_pool(name="dd", bufs=2) as dd,
            tc.tile_pool(name="qp", bufs=3) as qp,
            tc.tile_pool(name="asg", bufs=2) as asgp,
            tc.tile_pool(name="qft", bufs=1) as qftp,
            tc.tile_pool(name="dram", bufs=1, space="DRAM") as dramp,
            tc.tile_pool(name="mm_ps", bufs=4, space="PSUM") as mmps,
            tc.tile_pool(name="tp_ps", bufs=3, space="PSUM") as tpps,
        ):
            ident = const.tile([128, 128], dt.float32)
            make_identity(nc, ident[:])

            # ---------------- load all feat shards up front ----------------
            f_sbs = []
            for b in range(n_batch):
                f_sb = fp.tile([128, CC, HW], dt.float32, tag="f", name=f"f_sb{b}")
                nc.sync.dma_start(
                    out=f_sb[:],
                    in_=feat_ap[b].rearrange("(cc p) hw -> p cc hw", p=128),
                )
                f_sbs.append(f_sb)

            def batch_prologue(b):
                """f transpose (tokens, c) + token norms + softmax/dist scale vectors."""
                f_sb = f_sbs[b]
                fT = ftp.tile([TT, NJ, C], dt.float32, tag="fT", name=f"fT{b}")
                nsq = vecs.tile([TT, NJ], dt.float32, tag="nsq", name=f"nsq{b}")
                scr_f = small.tile([TT, C], dt.float32, tag="scrf", name=f"scrf{b}")
                for j in range(NJ):
                    ft_ps = tpps.tile([TT, C], dt.float32, tag="tp", name=f"ft_ps{b}_{j}")
                    for cc in range(CC):
                        nc.tensor.transpose(
                            ft_ps[:, cc * 128 : (cc + 1) * 128],
                            f_sb[:, cc, j * TT : (j + 1) * TT],
                            ident[:],
                        )
                    nc.scalar.copy(out=fT[:, j, :], in_=ft_ps[:])
                    nc.scalar.activation(
                        out=scr_f[:], in_=ft_ps[:], func=Act.Square,
                        accum_out=nsq[:, j : j + 1],
                    )
                nc.sync.dma_start(
                    out=f_out[b].rearrange("(j p) c -> p j c", p=TT), in_=fT[:]
                )

                rs_x = _rsqrt_newton(nc, vecs, nsq[:], TT, NJ)  # 1/nx
                sx = vecs.tile([TT, NJ], dt.float32, tag="sx", name=f"sx{b}")
                t1 = vecs.tile([TT, NJ], dt.float32, tag="t1", name=f"t1{b}")
                nc.vector.tensor_tensor(out=t1[:], in0=rs_x[:], in1=rs_x[:], op=Alu.mult)
                nc.vector.tensor_tensor(out=sx[:], in0=nsq[:], in1=t1[:], op=Alu.mult)
                s1 = vecs.tile([TT, NJ], dt.float32, tag="s1", name=f"s1{b}")   # -2/nx
                s2 = vecs.tile([TT, NJ], dt.float32, tag="s2", name=f"s2{b}")   # sx + 1
                sE = vecs.tile([TT, NJ], dt.float32, tag="sE", name=f"sE{b}")   # 20/nx
                bE = vecs.tile([TT, NJ], dt.float32, tag="bE", name=f"bE{b}")   # -10*sx
                nc.vector.tensor_scalar(out=s1[:], in0=rs_x[:], scalar1=-2.0, scalar2=None, op0=Alu.mult)
                nc.vector.tensor_scalar(out=s2[:], in0=sx[:], scalar1=1.0, scalar2=None, op0=Alu.add)
                nc.vector.tensor_scalar(out=sE[:], in0=rs_x[:], scalar1=20.0, scalar2=None, op0=Alu.mult)
                nc.vector.tensor_scalar(out=bE[:], in0=sx[:], scalar1=-10.0, scalar2=None, op0=Alu.mult)
                return {"s1": s1, "s2": s2, "sE": sE, "bE": bE}

            # batch 0 prologue first so the PE has transpose work immediately,
            # then codebook prep, then the remaining prologues
            bstate = {0: batch_prologue(0)}

            # ---------------- codebook prep (normalize in place) ----------------
            cb_sb = prep.tile([128, KC, C], dt.float32, tag="cb")
            nc.sync.dma_start(
                out=cb_sb[:], in_=cb[:, :].rearrange("(n p) c -> p n c", p=128)
            )
            ne2 = const.tile([128, KC], dt.float32)
            scr_cb = prep.tile([128, C], dt.float32, tag="scr")
            for n in range(KC):
                nc.scalar.activation(
                    out=scr_cb[:], in_=cb_sb[:, n, :], func=Act.Square,
                    accum_out=ne2[:, n : n + 1],
                )
            rs_e = _rsqrt_newton(nc, const, ne2[:], 128, KC)
            cn_sb = cb_sb
            for n in range(KC):
                nc.scalar.activation(
                    out=cn_sb[:, n, :], in_=cb_sb[:, n, :], func=Act.Identity,
                    scale=rs_e[:, n : n + 1],
                )
            # normalized codebook back to DRAM as the gather table
            cn_dram = dramp.tile([K, C], dt.float32)
            nc.sync.dma_start(
                out=cn_dram[:, :].rearrange("(n p) c -> p n c", p=128), in_=cn_sb[:]
            )
            # transposed codebook (c, K) for the PE: 3 chunks of (128, 2048)
            cnT = const.tile([128, CC, K], dt.float32)
            for n in range(KC):
                for cc in range(CC):
                    tp = tpps.tile([128, 128], dt.float32, tag="tp")
                    nc.tensor.transpose(
                        tp[:], cn_sb[:, n, cc * 128 : (cc + 1) * 128], ident[:]
                    )
                    nc.scalar.copy(out=cnT[:, cc, n * 128 : (n + 1) * 128], in_=tp[:])

            for b in range(1, n_batch):
                bstate[b] = batch_prologue(b)

            # ---------------- per batch main loops ----------------
            for b in range(n_batch):
                f_sb = f_sbs[b]
                s1, s2 = bstate[b]["s1"], bstate[b]["s2"]
                sE, bE = bstate[b]["sE"], bstate[b]["bE"]
                qfT = qftp.tile([128, CC, HW], dt.float32)

                def front(j):
                    js = slice(j * TT, (j + 1) * TT)
                    e_sb = ed.tile([TT, K], dt.float32, tag="e", name=f"e_sb{j}")
                    dist_sb = dd.tile([TT, K], dt.float32, tag="d", name=f"dist_sb{j}")
                    es4 = small.tile([TT, KT], dt.float32, tag="es4", name=f"es4_{j}")

                    m_ps = [
                        mmps.tile([TT, 512], dt.float32, tag="mm", name=f"m_ps{kt}")
                        for kt in range(KT)
                    ]
                    for cc in range(CC):
                        for kt in range(KT):
                            nc.tensor.matmul(
                                m_ps[kt][:],
                                lhsT=f_sb[:, cc, js],
                                rhs=cnT[:, cc, kt * 512 : (kt + 1) * 512],
                                start=(cc == 0),
                                stop=(cc == CC - 1),
                            )
                    for kt in range(KT):
                        ks = slice(kt * 512, (kt + 1) * 512)
                        nc.scalar.activation(
                            out=e_sb[:, ks], in_=m_ps[kt][:], func=Act.Exp,
                            scale=sE[:, j : j + 1], bias=bE[:, j : j + 1],
                            accum_out=es4[:, kt : kt + 1],
                        )
                        nc.scalar.activation(
                            out=dist_sb[:, ks], in_=m_ps[kt][:], func=Act.Identity,
                            scale=s1[:, j : j + 1], bias=s2[:, j : j + 1],
                        )
                    row0 = b * HW + j * TT
                    nc.sync.dma_start(out=dist_out[row0 : row0 + TT, :], in_=dist_sb[:])

                    # softmax denominator and in-place prob
                    esum = small.tile([TT, 1], dt.float32, tag="esum", name=f"esum{j}")
                    nc.vector.reduce_sum(out=esum[:], in_=es4[:], axis=mybir.AxisListType.X)
                    r = small.tile([TT, 1], dt.float32, tag="r", name=f"r{j}")
                    nc.vector.reciprocal(out=r[:], in_=esum[:])
                    nc.vector.tensor_scalar(
                        out=e_sb[:], in0=e_sb[:], scalar1=r[:, :1], scalar2=None, op0=Alu.mult
                    )

                    # argmin (first index, matching jnp.argmin) on dist
                    dmin = small.tile([TT, 1], dt.float32, tag="dmin", name=f"dmin{j}")
                    nc.vector.tensor_reduce(
                        out=dmin[:], in_=dist_sb[:], axis=mybir.AxisListType.X, op=Alu.min
                    )
                    dmin8 = small.tile([TT, 8], dt.float32, tag="dmin8", name=f"dmin8_{j}")
                    nc.vector.tensor_copy(out=dmin8[:], in_=dmin[:, :1].to_broadcast([TT, 8]))
                    idx8 = small.tile([TT, 8], dt.uint32, tag="idx8", name=f"idx8_{j}")
                    nc.vector.max_index(out=idx8[:], in_max=dmin8[:], in_values=dist_sb[:])

                    # gather nearest codes from DRAM
                    q_sb = qp.tile([TT, C], dt.float32, tag="q", name=f"q_sb{j}")
                    nc.gpsimd.indirect_dma_start(
                        out=q_sb[:],
                        out_offset=None,
                        in_=cn_dram[:, :],
                        in_offset=bass.IndirectOffsetOnAxis(ap=idx8[:, :1], axis=0),
                    )
                    return {"j": j, "js": js, "e_sb": e_sb, "q_sb": q_sb}

                asgT_box = [None]

                def back(st):
                    j, js, e_sb, q_sb = st["j"], st["js"], st["e_sb"], st["q_sb"]
                    qt_ps = tpps.tile([128, CC * TT], dt.float32, tag="tp", name=f"qt_ps{j}")
                    for cc in range(CC):
                        nc.tensor.transpose(
                            qt_ps[:, cc * TT : (cc + 1) * TT],
                            q_sb[:, cc * 128 : (cc + 1) * 128],
                            ident[:TT, :TT],
                        )
                    nc.scalar.copy(
                        out=qfT[:, :, js],
                        in_=qt_ps[:].rearrange("p (cc t) -> p cc t", cc=CC),
                    )

                    # prob transpose into (k, hw): 16 chunks in groups of 4
                    if j % 2 == 0:
                        asgT_box[0] = asgp.tile([128, KC, 2 * TT], dt.float32, tag="asgT", name=f"asgT{j}")
                    asgT = asgT_box[0]
                    half = (j % 2) * TT
                    for g in range(KT):
                        tp_ps = tpps.tile([128, 4 * TT], dt.float32, tag="tp", name=f"tp_ps{j}_{g}")
                        for u in range(4):
                            kc = 4 * g + u
                            nc.tensor.transpose(
                                tp_ps[:, u * TT : (u + 1) * TT],
                                e_sb[:, kc * 128 : (kc + 1) * 128],
                                ident[:TT, :TT],
                            )
                        nc.vector.tensor_copy(
                            out=asgT[:, 4 * g : 4 * g + 4, half : half + TT],
                            in_=tp_ps[:].rearrange("p (k t) -> p k t", k=4),
                        )
                    if j % 2 == 1 or j == NJ - 1:
                        jp0 = (j // 2) * 2 * TT
                        width = TT if j == NJ - 1 and NJ % 2 == 1 else 2 * TT
                        nc.sync.dma_start(
                            out=asg_out[b][:, jp0 : jp0 + width].rearrange(
                                "(kc p) t -> p kc t", p=128
                            ),
                            in_=asgT[:, :, :width],
                        )

                prev = None
                for j in range(NJ):
                    st = front(j)
                    if prev is not None:
                        back(prev)
                    prev = st
                back(prev)

                nc.sync.dma_start(
                    out=qf_out[b].rearrange("(cc p) hw -> p cc hw", p=128), in_=qfT[:]
                )

    nc.compile()
    return nc


def build_trivial_nc(n_batch=4):
    """Same external I/O as build_nc but near-zero work — used to calibrate
    per-call dispatch overhead when timing (bench.py)."""
    nc = bacc.Bacc("TRN2", target_bir_lowering=False, debug=False, num_devices=N_CORES)
    n_tok = n_batch * HW
    feat = nc.dram_tensor("feat", (n_batch, C, H, W), dt.float32, kind="ExternalInput")
    cb = nc.dram_tensor("codebook", (K, C), dt.float32, kind="ExternalInput")
    f_out = nc.dram_tensor("f_out", (n_batch, HW, C), dt.float32, kind="ExternalOutput")
    qf_out = nc.dram_tensor("qf_out", (n_batch, C, HW), dt.float32, kind="ExternalOutput")
    asg_out = nc.dram_tensor("asg_out", (n_batch, K, HW), dt.float32, kind="ExternalOutput")
    dist_out = nc.dram_tensor("dist_out", (n_tok, K), dt.float32, kind="ExternalOutput")
    with tile.TileContext(nc) as tc:
        with tc.tile_pool(name="p", bufs=1) as pool:
            t = pool.tile([128, 384], dt.float32)
            nc.sync.dma_start(out=t[:], in_=cb[0:128, :])
            nc.sync.dma_start(out=f_out[0, 0:128, :], in_=t[:])
            nc.sync.dma_start(out=qf_out[0, 0:128, 0:384], in_=t[:])
            nc.sync.dma_start(out=asg_out[0, 0:128, 0:384], in_=t[:])
            nc.sync.dma_start(out=dist_out[0:128, 0:384], in_=t[:])
    nc.compile()
    return nc


_NC_CACHE = {}


def _get_nc(n_batch):
    if n_batch not in _NC_CACHE:
        _NC_CACHE[n_batch] = build_nc(n_batch)
    return _NC_CACHE[n_batch]


def kernel(feat: np.ndarray, codebook: np.ndarray, _trace=False):
    feat = np.ascontiguousarray(np.asarray(feat, dtype=np.float32))
    codebook = np.ascontiguousarray(np.asarray(codebook, dtype=np.float32))
    b_full = feat.shape[0]
    bpc = b_full // N_CORES
    nc = _get_nc(bpc)

    in_maps = [
        {"feat": feat[m * bpc : (m + 1) * bpc], "codebook": codebook}
        for m in range(N_CORES)
    ]
    res = run_bass_kernel_spmd(nc, in_maps, list(range(N_CORES)), trace=_trace)

    f = np.concatenate([r["f_out"] for r in res.results], axis=0)
    qf = np.concatenate([r["qf_out"] for r in res.results], axis=0)
    asg = np.concatenate([r["asg_out"] for r in res.results], axis=0)
    dist = np.concatenate([r["dist_out"] for r in res.results], axis=0)

    f = f.reshape(b_full, H, W, C)
    qf = qf.reshape(b_full, C, H, W)
    asg = asg.reshape(b_full, K, H, W)
    out = (f, qf, asg, dist)
    if _trace:
        return out, res
    return out
